# revision 7
# baseline (speedup 1.0000x reference)
"""Group-wise Hadamard transform + symmetric int8 quantization for Trainium2.

Contract: kernel(x) with x [4, 4096, 4096] f32 returns
(scale [4,4096,32] f32, zero_point [4,4096,32] f32, x_int [4,4096,4096] f32),
matching reference.reference(x).

Strategy: data-parallel over 8 NeuronCores, 2048 tokens each. Per core:
  - per 128-token row-block, per group of 128 features:
      PE: transpose x-block via matmul-by-identity  -> psum_xt [i, tok]
      ACT: copy psum->sbuf with 1/sqrt(128) folded into the activation scale
      PE: matmul xt.T @ H -> psum_y [tok, j]  (y == normalized Hadamard coeffs)
      DVE: abs-max reduce per group -> absmax[tok, group]
      DVE: scale = max(absmax/127, 1e-8); rscale = 1/scale
      DVE: t = y * rscale (group-broadcast via stride-0 AP)
      DVE: q = int8((t + 1.5*2^23) - 1.5*2^23)   # exact round-to-nearest-even
  - DMA out q (int8) and scale (f32); host converts q -> f32.
"""

import sys
from contextlib import ExitStack

import numpy as np

sys.path.insert(0, "/opt/trn_rl_repo")

import concourse.bacc as bacc  # noqa: E402
import concourse.bass as bass  # noqa: E402
import concourse.tile as tile  # noqa: E402
from concourse import mybir  # noqa: E402
from concourse.bass_utils import run_bass_kernel_spmd  # noqa: E402

B, S, D = 4, 4096, 4096
G = 128                  # group size (hadamard + quant)
NG = D // G              # 32 groups
NCORES = 8
TOK = B * S              # 16384 tokens
TPC = TOK // NCORES      # 2048 tokens per core
P = 128                  # partitions / tokens per row-block
GPO = 8                  # groups per oct (psum tile = 8 groups = 1024 cols)
RSQRT_G = float(1.0 / np.sqrt(np.float32(G)))
MAGIC = 12582912.0       # 1.5 * 2**23: (v + MAGIC) - MAGIC == rint(v) for |v| < 2**22
F32 = mybir.dt.float32


def _hadamard(n: int) -> np.ndarray:
    h = np.array([[1.0]], dtype=np.float32)
    while h.shape[0] < n:
        h = np.block([[h, h], [h, -h]])
    return h.astype(np.float32)


def build(tpc: int = TPC, d: int = D) -> bass.Bass:
    """Build the per-core Bass program ([tpc, d] f32 in -> int8 q + f32 scale)."""
    ng = d // G
    rows = tpc // P
    octs = ng // GPO

    nc = bacc.Bacc("TRN2", debug=False)
    x_d = nc.dram_tensor("x", [tpc, d], F32, kind="ExternalInput")
    h_d = nc.dram_tensor("hmat", [G, G], F32, kind="ExternalInput")
    i_d = nc.dram_tensor("ident", [G, G], F32, kind="ExternalInput")
    q_d = nc.dram_tensor("q", [tpc, d], mybir.dt.int8, kind="ExternalOutput")
    s_d = nc.dram_tensor("scale", [tpc, ng], F32, kind="ExternalOutput")

    xd, qd, sd = x_d.ap(), q_d.ap(), s_d.ap()

    with tile.TileContext(nc) as tc, ExitStack() as ctx:
        consts = ctx.enter_context(tc.tile_pool(name="consts", bufs=1))
        xp = ctx.enter_context(tc.tile_pool(name="xp", bufs=3))
        xtp = ctx.enter_context(tc.tile_pool(name="xtp", bufs=3))
        psp = ctx.enter_context(tc.tile_pool(name="psp", bufs=4, space="PSUM"))
        stp = ctx.enter_context(tc.tile_pool(name="stp", bufs=3))
        tp = ctx.enter_context(tc.tile_pool(name="tp", bufs=2))
        qp = ctx.enter_context(tc.tile_pool(name="qp", bufs=2))

        h_sb = consts.tile([G, G], F32)
        nc.sync.dma_start(h_sb[:], h_d.ap())
        i_sb = consts.tile([G, G], F32)
        nc.sync.dma_start(i_sb[:], i_d.ap())

        for r in range(rows):
            rsl = slice(r * P, (r + 1) * P)
            x_sb = xp.tile([P, d], F32)
            nchunk = 4
            cw = d // nchunk
            for c in range(nchunk):
                nc.sync.dma_start(
                    x_sb[:, c * cw:(c + 1) * cw], xd[rsl, c * cw:(c + 1) * cw]
                )

            absmax = stp.tile([P, ng], F32, tag="absmax")
            scale_sb = stp.tile([P, ng], F32, tag="scale")
            rscale = stp.tile([P, ng], F32, tag="rscale")
            t_sb = tp.tile([P, d], F32)
            q_sb = qp.tile([P, d], mybir.dt.int8)

            for o in range(octs):
                osl = slice(o * GPO, (o + 1) * GPO)
                csl = slice(o * GPO * G, (o + 1) * GPO * G)

                ps_xt = psp.tile([P, GPO * G], F32, tag="ps")
                for k in range(GPO):
                    g = o * GPO + k
                    nc.tensor.matmul(
                        ps_xt[:, k * G:(k + 1) * G],
                        x_sb[:, g * G:(g + 1) * G],
                        i_sb[:],
                        start=True,
                        stop=True,
                    )
                xt_sb = xtp.tile([P, GPO * G], F32)
                nc.scalar.activation(
                    xt_sb[:], ps_xt[:], mybir.ActivationFunctionType.Copy,
                    scale=RSQRT_G,
                )

                ps_y = psp.tile([P, GPO * G], F32, tag="ps")
                for k in range(GPO):
                    nc.tensor.matmul(
                        ps_y[:, k * G:(k + 1) * G],
                        xt_sb[:, k * G:(k + 1) * G],
                        h_sb[:],
                        start=True,
                        stop=True,
                    )

                y3 = ps_y[:].rearrange("p (g j) -> p g j", j=G)
                nc.vector.tensor_reduce(
                    out=absmax[:, osl],
                    in_=y3,
                    axis=mybir.AxisListType.X,
                    op=mybir.AluOpType.max,
                    apply_absolute_value=True,
                )
                # scale = max(absmax * (1/127), 1e-8)  (ref divides; <=1ulp diff)
                nc.vector.tensor_scalar(
                    out=scale_sb[:, osl], in0=absmax[:, osl],
                    scalar1=float(np.float32(1.0) / np.float32(127.0)),
                    scalar2=1e-8,
                    op0=mybir.AluOpType.mult, op1=mybir.AluOpType.max,
                )
                nc.vector.reciprocal(rscale[:, osl], scale_sb[:, osl])

                r3 = rscale[:, osl].rearrange("p (g j) -> p g j", j=1)
                r3b = r3.to_broadcast([P, GPO, G])
                nc.vector.tensor_tensor(
                    out=t_sb[:, csl].rearrange("p (g j) -> p g j", j=G),
                    in0=y3,
                    in1=r3b,
                    op=mybir.AluOpType.mult,
                )

            # round-to-nearest-even + int8 convert (values already in [-127,127])
            nc.vector.tensor_scalar(
                out=q_sb[:], in0=t_sb[:],
                scalar1=MAGIC, scalar2=-MAGIC,
                op0=mybir.AluOpType.add, op1=mybir.AluOpType.add,
            )
            nc.sync.dma_start(qd[rsl, :], q_sb[:])
            nc.sync.dma_start(sd[rsl, :], scale_sb[:])

    nc.compile()
    return nc


_cached_nc = None


def _run(x: np.ndarray, **spmd_kwargs):
    global _cached_nc
    x = np.ascontiguousarray(np.asarray(x, dtype=np.float32))
    assert x.shape == (B, S, D), x.shape

    if _cached_nc is None:
        _cached_nc = build()
    nc = _cached_nc

    hmat = _hadamard(G)
    ident = np.eye(G, dtype=np.float32)

    xf = x.reshape(TOK, D)
    in_maps = [
        {
            "x": np.ascontiguousarray(xf[c * TPC:(c + 1) * TPC]),
            "hmat": hmat,
            "ident": ident,
        }
        for c in range(NCORES)
    ]
    res = run_bass_kernel_spmd(nc, in_maps, core_ids=list(range(NCORES)), **spmd_kwargs)

    q = np.concatenate([res.results[c]["q"] for c in range(NCORES)], axis=0)
    sc = np.concatenate([res.results[c]["scale"] for c in range(NCORES)], axis=0)

    x_int = q.astype(np.float32).reshape(B, S, D)
    scale = sc.reshape(B, S, NG)
    zero_point = np.zeros_like(scale)
    return (scale, zero_point, x_int), res


def kernel(x: np.ndarray):
    outs, _ = _run(x)
    return outs


# revision 27
# speedup vs baseline: 1.6939x; 1.6939x over previous
"""Group-wise Hadamard transform + symmetric int8 quantization for Trainium2.

Contract: kernel(x) with x [4, 4096, 4096] f32 returns
(scale [4,4096,32] f32, zero_point [4,4096,32] f32, x_int [4,4096,4096] f32),
matching reference.reference(x).

Strategy: data-parallel over 8 NeuronCores, 2048 tokens each. Per core:
  - host converts x to fp16 (or hi/lo bf16 limbs for near-fp32 accuracy);
    device input layout is the natural token-major [tpc, ng, 128] view
  - one DMA xbar-transpose per 512-token chunk loads ALL groups transposed:
    [512, 4096] -> SBUF [128 feat, 32 group, 512 tok] (contiguous source)
  - PE: y_u = x^T.T @ H per 128x128 block (H is +-1, exact in 16-bit; the
    1/sqrt(128) normalization is folded into the scale constants)
  - DVE: per-group abs-max reduce of y_u; t2 = max(absmax*(1/127), T0);
    rs2 = 1/t2  (so y_u*rs2 == y_normalized/scale)
  - ACT: scale_out = t2 * (1/sqrt(128))
  - DVE + ACT: q = int8(y_u * rs2) (HW fp32->int8 convert rounds to
    nearest even, verified on device)
  - chunk-batched DMA out of q (int8) and scale (f32); host converts q->f32.
"""

import sys
from contextlib import ExitStack

import numpy as np

sys.path.insert(0, "/opt/trn_rl_repo")

import concourse.bacc as bacc  # noqa: E402
import concourse.bass as bass  # noqa: E402
import concourse.tile as tile  # noqa: E402
from concourse import mybir  # noqa: E402
from concourse.bass_utils import run_bass_kernel_spmd  # noqa: E402

B, S, D = 4, 4096, 4096
G = 128                  # group size (hadamard + quant)
NG = D // G              # 32 groups
NCORES = 8
TOK = B * S              # 16384 tokens
TPC = TOK // NCORES      # 2048 tokens per core
P = 128                  # partitions / tokens per block
GPO = 8                  # groups per oct (psum tile = 8 groups = 1024 cols)
CHUNK = 512              # tokens per transpose chunk
F32 = mybir.dt.float32

MODE = "fp16"            # "fp16" (1 limb) or "bf16x2" (hi/lo limbs, ~fp32 acc)
ACT_GROUPS = 5           # groups per oct quantized on ScalarE (rest on VectorE)

RSQRT_G = float(np.float32(1.0) / np.float32(np.sqrt(np.float32(G))))
INV_QMAX = float(np.float32(1.0) / np.float32(127.0))
# t2 clamp: scale = max(absmax_n/127, 1e-8) with absmax_n = absmax_u/sqrt(g)
# => t2 = max(absmax_u/127, 1e-8*sqrt(g)); never hit for randn inputs.
T2_MIN = float(np.float32(1e-8) / np.float32(RSQRT_G))


def _hadamard(n: int) -> np.ndarray:
    h = np.array([[1.0]], dtype=np.float32)
    while h.shape[0] < n:
        h = np.block([[h, h], [h, -h]])
    return h.astype(np.float32)


def _limb_dtype():
    return mybir.dt.float16 if MODE == "fp16" else mybir.dt.bfloat16


def _nlimbs():
    return 1 if MODE == "fp16" else 2


def build(tpc: int = TPC, d: int = D) -> bass.Bass:
    """Per-core program: 16-bit x limbs in -> int8 q + f32 scale out."""
    ng = d // G
    nchunks = tpc // CHUNK
    ntb = CHUNK // P
    octs = ng // GPO
    ldt = _limb_dtype()
    nl = _nlimbs()
    dg = GPO - ACT_GROUPS  # leading groups per oct quantized on DVE

    nc = bacc.Bacc("TRN2", debug=False)
    x_ds = [
        nc.dram_tensor(f"x{li}", [tpc, ng, G], ldt, kind="ExternalInput")
        for li in range(nl)
    ]
    h_d = nc.dram_tensor("hmat", [G, G], ldt, kind="ExternalInput")
    q_d = nc.dram_tensor("q", [tpc, d], mybir.dt.int8, kind="ExternalOutput")
    s_d = nc.dram_tensor("scale", [tpc, ng], F32, kind="ExternalOutput")

    xds = [x.ap() for x in x_ds]
    qd = q_d.ap().rearrange("(c tb p) d -> c p tb d", tb=ntb, p=P)
    sd = s_d.ap().rearrange("(c tb p) g -> c p tb g", tb=ntb, p=P)

    with tile.TileContext(nc) as tc, ExitStack() as ctx:
        consts = ctx.enter_context(tc.tile_pool(name="consts", bufs=1))
        xtp = ctx.enter_context(tc.tile_pool(name="xtp", bufs=(3 if nl == 1 else 2)))
        psp = ctx.enter_context(tc.tile_pool(name="psp", bufs=4, space="PSUM"))
        stp = ctx.enter_context(tc.tile_pool(name="stp", bufs=4))
        qp = ctx.enter_context(tc.tile_pool(name="qp", bufs=2))
        scp = ctx.enter_context(tc.tile_pool(name="scp", bufs=2))

        h_sb = consts.tile([G, G], ldt)
        nc.sync.dma_start(h_sb[:], h_d.ap())

        for c in range(nchunks):
            tsl = slice(c * CHUNK, (c + 1) * CHUNK)
            # Two half-transposes per limb (one per HWDGE ring):
            # [CHUNK, 16*G] -> [G, 16, CHUNK] each.
            hg = ng // 2
            xtT = []
            for li in range(nl):
                t = xtp.tile([G, ng, CHUNK], ldt, tag="xtT")
                for half in (0, 1):
                    gsl = slice(half * hg, (half + 1) * hg)
                    nc.sync.dma_start_transpose(
                        t[:, gsl, :],
                        xds[li][tsl, gsl, :].rearrange("t g i -> t (g i)"),
                    )
                xtT.append(t)

            q_sb = qp.tile([P, ntb, d], mybir.dt.int8)
            sc_sb = scp.tile([P, ntb, ng], F32)

            for tb in range(ntb):
                bsl = slice(tb * P, (tb + 1) * P)

                absmax = stp.tile([P, ng], F32, tag="absmax")
                rs2 = stp.tile([P, ng], F32, tag="rs2")

                for pair in range(octs // 2):
                    y3s = {}
                    for o in (2 * pair, 2 * pair + 1):
                        osl = slice(o * GPO, (o + 1) * GPO)
                        ps_y = psp.tile([P, GPO * G], F32, tag="ps")
                        for k in range(GPO):
                            g = o * GPO + k
                            for li in range(nl):
                                nc.tensor.matmul(
                                    ps_y[:, k * G:(k + 1) * G],
                                    xtT[li][:, g, bsl],
                                    h_sb[:],
                                    start=(li == 0),
                                    stop=(li == nl - 1),
                                )
                        y3 = ps_y[:].rearrange("p (g j) -> p g j", j=G)
                        y3s[o] = y3
                        nc.vector.tensor_reduce(
                            out=absmax[:, osl],
                            in_=y3,
                            axis=mybir.AxisListType.X,
                            op=mybir.AluOpType.max,
                            apply_absolute_value=True,
                        )

                    # per-pair: t2 = max(absmax/127, T2_MIN); rs2 = 1/t2;
                    # scale_out = t2/sqrt(g)  (ACT, constant scale)
                    psl = slice(2 * pair * GPO, (2 * pair + 2) * GPO)
                    nc.vector.tensor_scalar(
                        out=rs2[:, psl], in0=absmax[:, psl],
                        scalar1=INV_QMAX, scalar2=T2_MIN,
                        op0=mybir.AluOpType.mult, op1=mybir.AluOpType.max,
                    )
                    nc.scalar.activation(
                        sc_sb[:, tb, psl], rs2[:, psl],
                        mybir.ActivationFunctionType.Copy, scale=RSQRT_G,
                    )
                    nc.vector.reciprocal(rs2[:, psl], rs2[:, psl])

                    for o in (2 * pair, 2 * pair + 1):
                        g0 = o * GPO
                        if dg > 0:
                            r3 = rs2[:, g0:g0 + dg].rearrange(
                                "p (g j) -> p g j", j=1
                            ).to_broadcast([P, dg, G])
                            nc.vector.tensor_tensor(
                                out=q_sb[:, tb, g0 * G:(g0 + dg) * G].rearrange(
                                    "p (g j) -> p g j", j=G
                                ),
                                in0=y3s[o][:, 0:dg, :],
                                in1=r3,
                                op=mybir.AluOpType.mult,
                            )
                        for k in range(dg, GPO):
                            g = g0 + k
                            nc.scalar.activation(
                                q_sb[:, tb, g * G:(g + 1) * G],
                                y3s[o][:, k, :],
                                mybir.ActivationFunctionType.Copy,
                                scale=rs2[:, g:g + 1],
                            )

            # chunk-batched outputs (alternate rings to dodge long transposes)
            nc.sync.dma_start(qd[c], q_sb[:])
            nc.sync.dma_start(sd[c], sc_sb[:])

    nc.compile()
    return nc


_cached_nc = None


def _limbs_np(xf: np.ndarray):
    """Split the [TOK, D] f32 array into 16-bit limb arrays per MODE."""
    if MODE == "fp16":
        return [xf.astype(np.float16)]
    import ml_dtypes

    hi = xf.astype(ml_dtypes.bfloat16)
    lo = (xf - hi.astype(np.float32)).astype(ml_dtypes.bfloat16)
    return [hi, lo]


def _run(x: np.ndarray, **spmd_kwargs):
    global _cached_nc
    x = np.ascontiguousarray(np.asarray(x, dtype=np.float32))
    assert x.shape == (B, S, D), x.shape

    if _cached_nc is None:
        _cached_nc = build()
    nc = _cached_nc

    hmat = _hadamard(G)
    if MODE == "fp16":
        hmat = hmat.astype(np.float16)
    else:
        import ml_dtypes

        hmat = hmat.astype(ml_dtypes.bfloat16)

    limbs = _limbs_np(x.reshape(TOK, D))
    in_maps = []
    for c in range(NCORES):
        m = {"hmat": hmat}
        for li, arr in enumerate(limbs):
            m[f"x{li}"] = arr[c * TPC:(c + 1) * TPC].reshape(TPC, NG, G)
        in_maps.append(m)
    res = run_bass_kernel_spmd(nc, in_maps, core_ids=list(range(NCORES)), **spmd_kwargs)

    q = np.concatenate([res.results[c]["q"] for c in range(NCORES)], axis=0)
    sc = np.concatenate([res.results[c]["scale"] for c in range(NCORES)], axis=0)

    x_int = q.astype(np.float32).reshape(B, S, D)
    scale = sc.reshape(B, S, NG)
    zero_point = np.zeros_like(scale)
    return (scale, zero_point, x_int), res


def kernel(x: np.ndarray):
    outs, _ = _run(x)
    return outs
